# revision 5
# baseline (speedup 1.0000x reference)
"""Trainium2 Bass kernel for CustomMHA (B=2, N=2048, D=1024, H=16, fp32 io).

Sharding: 8 cores = (batch b = core//4) x (head-group g = core%4, 4 heads each).
Each core computes, for its batch and its 4 heads:
    attn_out_heads @ Wout[rows of its heads]  ->  a partial [N, D] output.
Host sums the 4 partials per batch (Megatron-style row-parallel output).

v2: bf16 datapath.  Inputs/weights are cast to bf16 on the host; x is
transposed by the DMA XBAR (dma_start_transpose, 16x128 tiles) straight
from DRAM into SBUF [D, N] layout -- no PE transposes, no stage copies.
All matmuls run bf16 (PSUM accumulate f32).  Rationale: HW trace showed
f32r matmuls throttling from 0.55 to 1.23 ns/row under sustained load;
bf16 halves moving-operand bytes, LDWEIGHTS time and ACT probs writes.

Per-core pipeline:
  1. XBAR-transpose xq, xkv -> xT [D, N] slabs (DMA engines only).
  2. Projections: qT/kT in [d_head, N] (transposed) layout, v in [N, d_head]
     natural layout with per-head ones/zeros-padded columns (vpo) so the
     AV matmul also produces the softmax denominator row.
  3. Attention per head, scoresT orientation [key-part, query-free]:
     QK matmul -> PSUM, exp on ACT (scale=1/8 folded) -> probsT (bf16),
     AV matmul accumulates over key blocks.  Denominator row broadcast via
     a K=1 matmul with a ones column; normalize with one DVE multiply.
  4. Row-sharded Wout matmul -> partial output (f32 out).

PSUM layout (8 banks):
  psA  (2 slots x 2 banks): qk tiles (exp pipeline)
  psAV (1 slot  x 2 banks): AV accumulators
  psC  (1 slot  x 2 banks): background -- late q projections, denominator
        broadcast, output projection
"""

import sys

sys.path.insert(0, "/opt/trn_rl_repo")

import numpy as np

import concourse.bass as bass
import concourse.mybir as mybir
import concourse.tile as tile
from concourse import bacc
from concourse.bass_utils import run_bass_kernel_spmd

F32 = mybir.dt.float32
BF16 = mybir.dt.bfloat16
EXP = mybir.ActivationFunctionType.Exp

N = 2048  # sequence length
D = 1024  # model dim
HL = 4    # heads per core
O = HL * 64  # per-core projection width (256)
P = 128
NSLAB = 512          # rows per projection slab
NSLABS = N // NSLAB  # 4
IG = 1024            # attention query-column group
NJB = N // P         # 16 key blocks
DC = D // P          # 8 contraction chunks


def build():
    nc = bacc.Bacc("TRN2", debug=False, num_devices=8)
    xq = nc.dram_tensor("xq", [N, D], BF16, kind="ExternalInput").ap()
    xkv = nc.dram_tensor("xkv", [N, D], BF16, kind="ExternalInput").ap()
    wq = nc.dram_tensor("wq", [D, O], BF16, kind="ExternalInput").ap()
    wk = nc.dram_tensor("wk", [D, O], BF16, kind="ExternalInput").ap()
    wv = nc.dram_tensor("wv", [D, O], BF16, kind="ExternalInput").ap()
    wout = nc.dram_tensor("wout", [O, D], BF16, kind="ExternalInput").ap()
    out = nc.dram_tensor("out", [N, D], F32, kind="ExternalOutput").ap()

    with tile.TileContext(nc) as tc:
        with (
            tc.tile_pool(name="consts", bufs=1) as consts,
            tc.tile_pool(name="weights", bufs=1) as wpool,
            tc.tile_pool(name="xT", bufs=1) as xTpool,
            tc.tile_pool(name="proj", bufs=1) as projpool,
            tc.tile_pool(name="probs", bufs=6) as probspool,
            tc.tile_pool(name="bc", bufs=2) as bcpool,
            tc.tile_pool(name="ostage", bufs=2) as opool,
            tc.tile_pool(name="psA", bufs=2, space="PSUM") as psA,
            tc.tile_pool(name="psAV", bufs=1, space="PSUM") as psAV,
            tc.tile_pool(name="psC", bufs=1, space="PSUM") as psC,
        ):
            # ---- constants ----
            onesb = consts.tile([P, P], BF16)
            nc.vector.memset(onesb[:], 1.0)
            # pat: [1, 0, 0, ...] column pattern for vpo padding halves
            pat = consts.tile([P, 64], BF16)
            nc.vector.memset(pat[:, 0:1], 1.0)
            nc.vector.memset(pat[:, 1:64], 0.0)

            # ---- weights: straight bf16 DMA ----
            with nc.named_scope("weights"):
                wr = {}
                for name, w in (("wq", wq), ("wk", wk), ("wv", wv)):
                    wt = wpool.tile([P, DC, O], BF16, tag=f"{name}r", name="wt")
                    nc.sync.dma_start(wt[:], w.rearrange("(c p) o -> p c o", p=P))
                    wr[name] = wt
                woutr = wpool.tile([P, 2, D], BF16, tag="woutr")
                nc.sync.dma_start(woutr[:], wout.rearrange("(c p) o -> p c o", p=P))

            # ---- persistent activations ----
            # per-slab xT tiles: the XBAR transpose needs a CONTIGUOUS SBUF
            # destination (strided dst produces wrong output / crashes on HW)
            xslab = {
                (chain, s): xTpool.tile(
                    [P, DC, NSLAB], BF16, tag=f"x{chain}{s}", name=f"x{chain}{s}"
                )
                for chain in ("kv", "q")
                for s in range(NSLABS)
            }
            qpT = projpool.tile([P, 2, N], BF16, tag="qpT")
            kpT = projpool.tile([P, 2, N], BF16, tag="kpT")
            vpo = [
                projpool.tile([P, NJB, P], BF16, tag=f"vpo{h}", name=f"vpo{h}")
                for h in range(HL)
            ]
            attT = projpool.tile([P, 2, N], BF16, tag="attT")

            # vpo padding halves: ones column + zeros
            for h in range(HL):
                pad0 = 64 if h % 2 == 0 else 0
                nc.vector.tensor_copy(
                    vpo[h][:, :, pad0 : pad0 + 64],
                    pat[:, None, :].to_broadcast([P, NJB, 64]),
                )

            def emit_xbar(chain, s):
                """XBAR-transpose one 512-row slab of x straight from DRAM."""
                xin = xkv if chain == "kv" else xq
                ssl = slice(s * NSLAB, (s + 1) * NSLAB)
                nc.sync.dma_start_transpose(xslab[chain, s][:], xin[ssl, :])

            _proj_flip = [0]
            _proj_rot = [0]

            def proj_psum(width):
                """Rotate projection accumulators through psA(2)/psAV/psC for a
                4-deep accumulate/copy pipeline (pre-attention only)."""
                _proj_rot[0] = (_proj_rot[0] + 1) % 4
                pool, tag = [(psA, "qk"), (psAV, "av"), (psA, "qk"), (psC, "c")][
                    _proj_rot[0]
                ]
                t = pool.tile([P, IG], F32, tag=tag, name="pp")
                return t[:, :width]

            def emit_slab(chain, s, background=False):
                """Project one 512-row slab of xT.

                background=True (late q slabs): run everything through the
                psC slot so the attention pipeline's psA/psAV rotations are
                never stalled; this work crawls along during attention.
                """
                cp = nc.any.tensor_copy
                xT = xslab[chain, s]
                ssl = slice(s * NSLAB, (s + 1) * NSLAB)
                wname, dstT = ("wk", kpT) if chain == "kv" else ("wq", qpT)
                for oc in range(2):
                    if background:
                        ps = psC.tile([P, IG], F32, tag="c", name="ps")[:, :NSLAB]
                    else:
                        ps = proj_psum(NSLAB)
                    for dc in range(DC):
                        nc.tensor.matmul(
                            ps[:],
                            wr[wname][:, dc, oc * P : (oc + 1) * P],
                            xT[:, dc, :],
                            start=(dc == 0),
                            stop=(dc == DC - 1),
                        )
                    cp(dstT[:, oc, ssl], ps[:])
                if chain == "kv":
                    # v projection (natural layout) + scatter into vpo
                    for half in range(NSLAB // P):
                        jb = s * (NSLAB // P) + half
                        if background:
                            ps = psC.tile([P, IG], F32, tag="c", name="ps")[:, :O]
                        else:
                            ps = proj_psum(O)
                        for dc in range(DC):
                            nc.tensor.matmul(
                                ps[:],
                                xT[:, dc, half * P : (half + 1) * P],
                                wr["wv"][:, dc, :],
                                start=(dc == 0),
                                stop=(dc == DC - 1),
                            )
                        for h in range(HL):
                            v0 = 0 if h % 2 == 0 else 64
                            cp(
                                vpo[h][:, jb, v0 : v0 + 64],
                                ps[:, h * 64 : (h + 1) * 64],
                            )

            def flush_av(carry):
                """Emit the deferred last AV pair of the previous group."""
                ph, pav, ppT = carry
                with tc.high_priority(offset=-30):
                    for nb in range(IG // 512):
                        nc.tensor.matmul(
                            pav[:, nb * 512 : (nb + 1) * 512],
                            vpo[ph][:, NJB - 1, :],
                            ppT[:, nb * 512 : (nb + 1) * 512],
                            start=False,
                            stop=True,
                        )

            def emit_attention(h, ig, at_jb0=None):
                oc, row0 = h // 2, (h % 2) * 64
                i0 = ig * IG
                av = psAV.tile([P, IG], F32, tag="av", name="av")
                # AV(jb) is emitted after QK(jb+1)/exp(jb+1) so the PE finishes
                # QK(jb+1) while exp(jb) runs; the final AV pair is carried into
                # the next group (flushed via at_jb0) so it never delays the
                # boundary exps.
                pend_pT = None
                for jb in range(NJB):
                    qk = psA.tile([P, IG], F32, tag="qk", name="qk")
                    for nb in range(IG // 512):
                        nc.tensor.matmul(
                            qk[:, nb * 512 : (nb + 1) * 512],
                            kpT[row0 : row0 + 64, oc, jb * P : (jb + 1) * P],
                            qpT[row0 : row0 + 64, oc, i0 + nb * 512 : i0 + (nb + 1) * 512],
                            start=True,
                            stop=True,
                        )
                    pT = probspool.tile([P, IG], BF16, tag="probsT", name="pT")
                    with nc.allow_low_precision(reason="bf16 probs"):
                        nc.scalar.activation(pT[:], qk[:], EXP, scale=0.125)
                    if jb == 0 and at_jb0 is not None:
                        at_jb0()
                    if pend_pT is not None:
                        pjb, ppT = pend_pT
                        with tc.high_priority(offset=-30):
                            for nb in range(IG // 512):
                                nc.tensor.matmul(
                                    av[:, nb * 512 : (nb + 1) * 512],
                                    vpo[h][:, pjb, :],
                                    ppT[:, nb * 512 : (nb + 1) * 512],
                                    start=(pjb == 0),
                                    stop=False,
                                )
                    pend_pT = (jb, pT)
                return av, (h, av, pend_pT[1])

            def emit_drain(h, ig, av):
                """Normalize group (h, ig); emitted one group late so the
                latency hides under the next group's j-loop.  Copy-first so
                the av PSUM slot is released after two DVE ops, then divide
                in place with the broadcast denominator still in PSUM."""
                vrow0 = (h % 2) * 64
                srow = 64 - vrow0
                i0 = ig * IG
                dst = attT[vrow0 : vrow0 + 64, h // 2, i0 : i0 + IG]
                with nc.allow_low_precision(reason="bf16 attention out"):
                    nc.vector.tensor_copy(dst, av[vrow0 : vrow0 + 64, :])
                    bc = bcpool.tile([P, IG], BF16, tag="bc", name="bc")
                    nc.vector.reciprocal(bc[srow : srow + 1, :], av[srow : srow + 1, :])
                    bcp = psC.tile([P, IG], F32, tag="c", name="bcp")
                    for nb in range(IG // 512):
                        nc.tensor.matmul(
                            bcp[:, nb * 512 : (nb + 1) * 512],
                            onesb[srow : srow + 1, :],
                            bc[srow : srow + 1, nb * 512 : (nb + 1) * 512],
                            start=True,
                            stop=True,
                        )
                    nc.vector.tensor_tensor(
                        dst, dst, bcp[vrow0 : vrow0 + 64, :], mybir.AluOpType.mult
                    )

            def emit_wout(ib, pool, tag, early=False):
                fin = pool.tile([P, D], F32, tag=tag, name="fin")
                for pc in range(2):
                    for nb in range(2):
                        nc.tensor.matmul(
                            fin[:, nb * 512 : (nb + 1) * 512],
                            attT[:, pc, ib * P : (ib + 1) * P],
                            woutr[:, pc, nb * 512 : (nb + 1) * 512],
                            start=(pc == 0),
                            stop=(pc == 1),
                        )
                ot = opool.tile([P, D], F32, tag="ostage", name="ot")
                cpf = nc.vector.tensor_copy if early else (
                    nc.scalar.copy if ib % 2 == 0 else nc.vector.tensor_copy
                )
                cpf(ot[:], fin[:])
                nc.sync.dma_start(out[ib * P : (ib + 1) * P, :], ot[:])

            # ---- emission order ----
            with nc.named_scope("proj"):
                for s in range(NSLABS):
                    emit_xbar("kv", s)
                emit_xbar("q", 0)
                emit_xbar("q", 1)
                for s in range(NSLABS):
                    emit_slab("kv", s)
                emit_xbar("q", 2)
                emit_xbar("q", 3)
                emit_slab("q", 0)
                emit_slab("q", 1)

            with nc.named_scope("attention"):
                groups = [(h, 0) for h in range(HL)] + [(h, 1) for h in range(HL)]
                carry = None
                pend_drain = None
                for gi, (h, ig) in enumerate(groups):
                    pc, pd = carry, pend_drain

                    def at_jb0(pc=pc, pd=pd):
                        if pc is not None:
                            flush_av(pc)
                        if pd is not None:
                            emit_drain(*pd)

                    av, carry = emit_attention(h, ig, at_jb0)
                    pend_drain = (h, ig, av)
                    if gi == 0:
                        with nc.named_scope("proj2"):
                            emit_slab("q", 2, background=True)
                    elif gi == 1:
                        with nc.named_scope("proj3"):
                            emit_slab("q", 3, background=True)
                    elif gi == 5:
                        # ig=0 halves of attT are final: first 8 output blocks
                        # crawl through the psC slot during ig=1 attention
                        with nc.named_scope("wout_early"), tc.high_priority(offset=-(10**6)):
                            for ib in range(N // P // 2):
                                emit_wout(ib, psC, "c", early=True)
                flush_av(carry)
                emit_drain(*pend_drain)

            # ---- output projection (second half) ----
            with nc.named_scope("wout"):
                rot = [(psA, "qk"), (psAV, "av"), (psA, "qk"), (psC, "c")]
                for ib in range(N // P // 2, N // P):
                    pool, tag = rot[ib % 4]
                    emit_wout(ib, pool, tag)

    nc.compile()
    return nc


_NC = None


def _get_nc():
    global _NC
    if _NC is None:
        _NC = build()
    return _NC


def make_in_maps(q, kv, Wq, Wkv, Wout):
    import ml_dtypes

    bf16 = ml_dtypes.bfloat16
    q = np.asarray(q, dtype=np.float32).astype(bf16)
    kv = np.asarray(kv, dtype=np.float32).astype(bf16)
    Wq = np.asarray(Wq, dtype=np.float32).astype(bf16)
    Wkv = np.asarray(Wkv, dtype=np.float32).astype(bf16)
    Wout = np.asarray(Wout, dtype=np.float32).astype(bf16)
    in_maps = []
    for c in range(8):
        b, g = c // 4, c % 4
        sl = slice(g * O, (g + 1) * O)
        in_maps.append(
            {
                "xq": np.ascontiguousarray(q[b]),
                "xkv": np.ascontiguousarray(kv[b]),
                "wq": np.ascontiguousarray(Wq[:, sl]),
                "wk": np.ascontiguousarray(Wkv[:, sl]),
                "wv": np.ascontiguousarray(Wkv[:, D + g * O : D + (g + 1) * O]),
                "wout": np.ascontiguousarray(Wout[sl, :]),
            }
        )
    return in_maps


def gather(results):
    out = np.zeros((2, N, D), dtype=np.float32)
    for c in range(8):
        out[c // 4] += results[c]["out"]
    return out


def kernel(**inputs):
    nc = _get_nc()
    in_maps = make_in_maps(**inputs)
    res = run_bass_kernel_spmd(nc, in_maps, core_ids=list(range(8)))
    return gather(res.results)


if __name__ == "__main__":
    rng = np.random.default_rng(0)
    ins = {
        "q": rng.standard_normal((2, N, D), dtype=np.float32),
        "kv": rng.standard_normal((2, N, D), dtype=np.float32),
        "Wq": (rng.standard_normal((D, D), dtype=np.float32) / np.sqrt(D)).astype(np.float32),
        "Wkv": (rng.standard_normal((D, 2 * D), dtype=np.float32) / np.sqrt(D)).astype(np.float32),
        "Wout": (rng.standard_normal((D, D), dtype=np.float32) / np.sqrt(D)).astype(np.float32),
    }
    out = kernel(**ins)
    print("ok", out.shape, out.dtype)


# revision 9
# speedup vs baseline: 1.5167x; 1.5167x over previous
"""Trainium2 Bass kernel for CustomMHA (B=2, N=2048, D=1024, H=16, fp32 io).

Sharding: 8 cores = (batch b = core//4) x (head-group g = core%4, 4 heads each).
Each core computes, for its batch and its 4 heads:
    attn_out_heads @ Wout[rows of its heads]  ->  a partial [N, D] output.
Host sums the 4 partials per batch (Megatron-style row-parallel output).

v2: bf16 datapath.  Inputs/weights are cast to bf16 on the host; x is
transposed by the DMA XBAR (dma_start_transpose, 16x128 tiles) straight
from DRAM into SBUF [D, N] layout -- no PE transposes, no stage copies.
All matmuls run bf16 (PSUM accumulate f32).  Rationale: HW trace showed
f32r matmuls throttling from 0.55 to 1.23 ns/row under sustained load;
bf16 halves moving-operand bytes, LDWEIGHTS time and ACT probs writes.

Per-core pipeline:
  1. XBAR-transpose xq, xkv -> xT [D, N] slabs (DMA engines only).
  2. Projections: qT/kT in [d_head, N] (transposed) layout, v in [N, d_head]
     natural layout with per-head ones/zeros-padded columns (vpo) so the
     AV matmul also produces the softmax denominator row.
  3. Attention per head, scoresT orientation [key-part, query-free]:
     QK matmul -> PSUM, exp on ACT (scale=1/8 folded) -> probsT (bf16),
     AV matmul accumulates over key blocks.  Denominator row broadcast via
     a K=1 matmul with a ones column; normalize with one DVE multiply.
  4. Row-sharded Wout matmul -> partial output (f32 out).

PSUM layout (8 banks):
  psA  (2 slots x 2 banks): qk tiles (exp pipeline)
  psAV (1 slot  x 2 banks): AV accumulators
  psC  (1 slot  x 2 banks): background -- late q projections, denominator
        broadcast, output projection
"""

import sys

sys.path.insert(0, "/opt/trn_rl_repo")

import numpy as np

import concourse.bass as bass
import concourse.mybir as mybir
import concourse.tile as tile
from concourse import bacc
from concourse.bass_utils import run_bass_kernel_spmd

F32 = mybir.dt.float32
BF16 = mybir.dt.bfloat16
EXP = mybir.ActivationFunctionType.Exp

N = 2048  # sequence length
D = 1024  # model dim
HL = 4    # heads per core
O = HL * 64  # per-core projection width (256)
P = 128
NSLAB = 512          # rows per projection slab
NSLABS = N // NSLAB  # 4
IG = 1024            # attention query-column group
NJB = N // P         # 16 key blocks
DC = D // P          # 8 contraction chunks


def build():
    nc = bacc.Bacc("TRN2", debug=False, num_devices=8)
    xq = nc.dram_tensor("xq", [N, D], BF16, kind="ExternalInput").ap()
    xkv = nc.dram_tensor("xkv", [N, D], BF16, kind="ExternalInput").ap()
    wq = nc.dram_tensor("wq", [D, O], BF16, kind="ExternalInput").ap()
    wk = nc.dram_tensor("wk", [D, O], BF16, kind="ExternalInput").ap()
    wv = nc.dram_tensor("wv", [D, O], BF16, kind="ExternalInput").ap()
    wout = nc.dram_tensor("wout", [O, D], BF16, kind="ExternalInput").ap()
    out = nc.dram_tensor("out", [N, D], F32, kind="ExternalOutput").ap()

    with tile.TileContext(nc) as tc:
        with (
            tc.tile_pool(name="consts", bufs=1) as consts,
            tc.tile_pool(name="weights", bufs=1) as wpool,
            tc.tile_pool(name="xT", bufs=1) as xTpool,
            tc.tile_pool(name="proj", bufs=1) as projpool,
            tc.tile_pool(name="probs", bufs=6) as probspool,
            tc.tile_pool(name="bc", bufs=2) as bcpool,
            tc.tile_pool(name="ostage", bufs=2) as opool,
            tc.tile_pool(name="psA", bufs=2, space="PSUM") as psA,
            tc.tile_pool(name="psAV", bufs=1, space="PSUM") as psAV,
            tc.tile_pool(name="psC", bufs=1, space="PSUM") as psC,
        ):
            # ---- constants ----
            onesb = consts.tile([P, P], BF16)
            nc.vector.memset(onesb[:], 1.0)
            # pat: [1, 0, 0, ...] column pattern for vpo padding halves
            pat = consts.tile([P, 64], BF16)
            nc.vector.memset(pat[:, 0:1], 1.0)
            nc.vector.memset(pat[:, 1:64], 0.0)

            # ---- weights: straight bf16 DMA ----
            with nc.named_scope("weights"):
                wr = {}
                for name, w in (("wq", wq), ("wk", wk), ("wv", wv)):
                    wt = wpool.tile([P, DC, O], BF16, tag=f"{name}r", name="wt")
                    nc.sync.dma_start(wt[:], w.rearrange("(c p) o -> p c o", p=P))
                    wr[name] = wt
                woutr = wpool.tile([P, 2, D], BF16, tag="woutr")
                nc.sync.dma_start(woutr[:], wout.rearrange("(c p) o -> p c o", p=P))

            # ---- persistent activations ----
            # per-slab xT tiles: the XBAR transpose needs a CONTIGUOUS SBUF
            # destination (strided dst produces wrong output / crashes on HW)
            xslab = {
                (chain, s): xTpool.tile(
                    [P, DC, NSLAB], BF16, tag=f"x{chain}{s}", name=f"x{chain}{s}"
                )
                for chain in ("kv", "q")
                for s in range(NSLABS)
            }
            qpT = projpool.tile([P, 2, N], BF16, tag="qpT")
            kpT = projpool.tile([P, 2, N], BF16, tag="kpT")
            vpo = [
                projpool.tile([P, NJB, P], BF16, tag=f"vpo{h}", name=f"vpo{h}")
                for h in range(HL)
            ]
            attT = projpool.tile([P, 2, N], BF16, tag="attT")

            # vpo padding halves: ones column + zeros
            for h in range(HL):
                pad0 = 64 if h % 2 == 0 else 0
                nc.vector.tensor_copy(
                    vpo[h][:, :, pad0 : pad0 + 64],
                    pat[:, None, :].to_broadcast([P, NJB, 64]),
                )

            def emit_xbar(chain, s):
                """XBAR-transpose one 512-row slab of x straight from DRAM."""
                xin = xkv if chain == "kv" else xq
                ssl = slice(s * NSLAB, (s + 1) * NSLAB)
                nc.sync.dma_start_transpose(xslab[chain, s][:], xin[ssl, :])

            _proj_flip = [0]
            _proj_rot = [0]

            def proj_psum(width):
                """Rotate projection accumulators through psA(2)/psAV/psC for a
                4-deep accumulate/copy pipeline (pre-attention only)."""
                _proj_rot[0] = (_proj_rot[0] + 1) % 4
                pool, tag = [(psA, "qk"), (psAV, "av"), (psA, "qk"), (psC, "c")][
                    _proj_rot[0]
                ]
                t = pool.tile([P, IG], F32, tag=tag, name="pp")
                return t[:, :width]

            def emit_slab(chain, s, background=False):
                """Project one 512-row slab of xT.

                background=True (late q slabs): run everything through the
                psC slot so the attention pipeline's psA/psAV rotations are
                never stalled; this work crawls along during attention.
                """
                cp = nc.any.tensor_copy
                xT = xslab[chain, s]
                ssl = slice(s * NSLAB, (s + 1) * NSLAB)
                wname, dstT = ("wk", kpT) if chain == "kv" else ("wq", qpT)
                for oc in range(2):
                    if background:
                        ps = psC.tile([P, IG], F32, tag="c", name="ps")[:, :NSLAB]
                    else:
                        ps = proj_psum(NSLAB)
                    for dc in range(DC):
                        nc.tensor.matmul(
                            ps[:],
                            wr[wname][:, dc, oc * P : (oc + 1) * P],
                            xT[:, dc, :],
                            start=(dc == 0),
                            stop=(dc == DC - 1),
                        )
                    cp(dstT[:, oc, ssl], ps[:])
                if chain == "kv":
                    # v projection (natural layout) + scatter into vpo
                    for half in range(NSLAB // P):
                        jb = s * (NSLAB // P) + half
                        if background:
                            ps = psC.tile([P, IG], F32, tag="c", name="ps")[:, :O]
                        else:
                            ps = proj_psum(O)
                        for dc in range(DC):
                            nc.tensor.matmul(
                                ps[:],
                                xT[:, dc, half * P : (half + 1) * P],
                                wr["wv"][:, dc, :],
                                start=(dc == 0),
                                stop=(dc == DC - 1),
                            )
                        for h in range(HL):
                            v0 = 0 if h % 2 == 0 else 64
                            cp(
                                vpo[h][:, jb, v0 : v0 + 64],
                                ps[:, h * 64 : (h + 1) * 64],
                            )

            def flush_av(carry):
                """Emit the deferred last AV pair of the previous group."""
                ph, pav, ppT = carry
                with tc.high_priority(offset=-30):
                    for nb in range(IG // 512):
                        nc.tensor.matmul(
                            pav[:, nb * 512 : (nb + 1) * 512],
                            vpo[ph][:, NJB - 1, :],
                            ppT[:, nb * 512 : (nb + 1) * 512],
                            start=False,
                            stop=True,
                        )

            def emit_attention(h, ig, at_jb=None):
                oc, row0 = h // 2, (h % 2) * 64
                i0 = ig * IG
                av = psAV.tile([P, IG], F32, tag="av", name="av")
                # AV(jb) is emitted after QK(jb+1)/exp(jb+1) so the PE finishes
                # QK(jb+1) while exp(jb) runs; the final AV pair is carried into
                # the next group (flushed via at_jb) so it never delays the
                # boundary exps.
                pend_pT = None
                for jb in range(NJB):
                    qk = psA.tile([P, IG], F32, tag="qk", name="qk")
                    for nb in range(IG // 512):
                        nc.tensor.matmul(
                            qk[:, nb * 512 : (nb + 1) * 512],
                            kpT[row0 : row0 + 64, oc, jb * P : (jb + 1) * P],
                            qpT[row0 : row0 + 64, oc, i0 + nb * 512 : i0 + (nb + 1) * 512],
                            start=True,
                            stop=True,
                        )
                    pT = probspool.tile([P, IG], BF16, tag="probsT", name="pT")
                    with nc.allow_low_precision(reason="bf16 probs"):
                        nc.scalar.activation(pT[:], qk[:], EXP, scale=0.125)
                    if at_jb is not None and jb in at_jb:
                        at_jb[jb]()
                    if pend_pT is not None:
                        pjb, ppT = pend_pT
                        with tc.high_priority(offset=-30):
                            for nb in range(IG // 512):
                                nc.tensor.matmul(
                                    av[:, nb * 512 : (nb + 1) * 512],
                                    vpo[h][:, pjb, :],
                                    ppT[:, nb * 512 : (nb + 1) * 512],
                                    start=(pjb == 0),
                                    stop=False,
                                )
                    pend_pT = (jb, pT)
                return av, (h, av, pend_pT[1])

            def emit_drain(h, ig, av):
                """Drain part 1: two quick copies so the av PSUM slot is
                released ~1.3us after the flush -- the next group's AV
                accumulation reuses it almost immediately.  The expensive
                reciprocal happens later in emit_drain2."""
                vrow0 = (h % 2) * 64
                srow = 64 - vrow0
                i0 = ig * IG
                dst = attT[vrow0 : vrow0 + 64, h // 2, i0 : i0 + IG]
                den = bcpool.tile([P, IG], F32, tag="den", name="den")
                with nc.allow_low_precision(reason="bf16 attention out"):
                    nc.scalar.copy(dst, av[vrow0 : vrow0 + 64, :])
                    nc.vector.tensor_copy(
                        den[srow : srow + 1, :], av[srow : srow + 1, :]
                    )
                return (h, ig, den)

            def emit_drain2(h, ig, den):
                """Drain part 2: reciprocal (6.5us on DVE, off the critical
                path), K=1 broadcast matmul, in-place normalize."""
                vrow0 = (h % 2) * 64
                srow = 64 - vrow0
                i0 = ig * IG
                dst = attT[vrow0 : vrow0 + 64, h // 2, i0 : i0 + IG]
                with nc.allow_low_precision(reason="bf16 attention out"):
                    bc = bcpool.tile([P, IG], BF16, tag="bc", name="bc")
                    nc.vector.reciprocal(
                        bc[srow : srow + 1, :], den[srow : srow + 1, :]
                    )
                    bcp = psC.tile([P, IG], F32, tag="c", name="bcp")
                    for nb in range(IG // 512):
                        nc.tensor.matmul(
                            bcp[:, nb * 512 : (nb + 1) * 512],
                            onesb[srow : srow + 1, :],
                            bc[srow : srow + 1, nb * 512 : (nb + 1) * 512],
                            start=True,
                            stop=True,
                        )
                    nc.vector.tensor_tensor(
                        dst, dst, bcp[vrow0 : vrow0 + 64, :], mybir.AluOpType.mult
                    )

            def emit_wout(ib, pool, tag, early=False):
                fin = pool.tile([P, D], F32, tag=tag, name="fin")
                for pc in range(2):
                    for nb in range(2):
                        nc.tensor.matmul(
                            fin[:, nb * 512 : (nb + 1) * 512],
                            attT[:, pc, ib * P : (ib + 1) * P],
                            woutr[:, pc, nb * 512 : (nb + 1) * 512],
                            start=(pc == 0),
                            stop=(pc == 1),
                        )
                ot = opool.tile([P, D], F32, tag="ostage", name="ot")
                cpf = nc.vector.tensor_copy if early else (
                    nc.scalar.copy if ib % 2 == 0 else nc.vector.tensor_copy
                )
                cpf(ot[:], fin[:])
                nc.sync.dma_start(out[ib * P : (ib + 1) * P, :], ot[:])

            # ---- emission order ----
            with nc.named_scope("proj"):
                for s in range(NSLABS):
                    emit_xbar("kv", s)
                emit_xbar("q", 0)
                emit_xbar("q", 1)
                for s in range(NSLABS):
                    emit_slab("kv", s)
                emit_xbar("q", 2)
                emit_xbar("q", 3)
                emit_slab("q", 0)
                emit_slab("q", 1)

            with nc.named_scope("attention"):
                groups = [(h, 0) for h in range(HL)] + [(h, 1) for h in range(HL)]
                carry = None
                pend_drain = None
                pend_d2 = [None]
                for gi, (h, ig) in enumerate(groups):
                    pc, pd = carry, pend_drain

                    def at_jb0(pc=pc, pd=pd):
                        if pc is not None:
                            flush_av(pc)
                        if pd is not None:
                            pend_d2[0] = emit_drain(*pd)

                    def at_jb6():
                        if pend_d2[0] is not None:
                            emit_drain2(*pend_d2[0])
                            pend_d2[0] = None

                    av, carry = emit_attention(h, ig, {0: at_jb0, 6: at_jb6})
                    pend_drain = (h, ig, av)
                    if gi == 0:
                        with nc.named_scope("proj2"):
                            emit_slab("q", 2, background=True)
                    elif gi == 1:
                        with nc.named_scope("proj3"):
                            emit_slab("q", 3, background=True)
                    elif gi == 5:
                        # ig=0 halves of attT are final: first 8 output blocks
                        # crawl through the psC slot during ig=1 attention
                        with nc.named_scope("wout_early"), tc.high_priority(offset=-(10**6)):
                            for ib in range(N // P // 2):
                                emit_wout(ib, psC, "c", early=True)
                flush_av(carry)
                emit_drain2(*emit_drain(*pend_drain))

            # ---- output projection (second half) ----
            with nc.named_scope("wout"):
                rot = [(psA, "qk"), (psAV, "av"), (psA, "qk"), (psC, "c")]
                for ib in range(N // P // 2, N // P):
                    pool, tag = rot[ib % 4]
                    emit_wout(ib, pool, tag)

    nc.compile()
    return nc


_NC = None


def _get_nc():
    global _NC
    if _NC is None:
        _NC = build()
    return _NC


def make_in_maps(q, kv, Wq, Wkv, Wout):
    import ml_dtypes

    bf16 = ml_dtypes.bfloat16
    q = np.asarray(q, dtype=np.float32).astype(bf16)
    kv = np.asarray(kv, dtype=np.float32).astype(bf16)
    Wq = np.asarray(Wq, dtype=np.float32).astype(bf16)
    Wkv = np.asarray(Wkv, dtype=np.float32).astype(bf16)
    Wout = np.asarray(Wout, dtype=np.float32).astype(bf16)
    in_maps = []
    for c in range(8):
        b, g = c // 4, c % 4
        sl = slice(g * O, (g + 1) * O)
        in_maps.append(
            {
                "xq": np.ascontiguousarray(q[b]),
                "xkv": np.ascontiguousarray(kv[b]),
                "wq": np.ascontiguousarray(Wq[:, sl]),
                "wk": np.ascontiguousarray(Wkv[:, sl]),
                "wv": np.ascontiguousarray(Wkv[:, D + g * O : D + (g + 1) * O]),
                "wout": np.ascontiguousarray(Wout[sl, :]),
            }
        )
    return in_maps


def gather(results):
    out = np.zeros((2, N, D), dtype=np.float32)
    for c in range(8):
        out[c // 4] += results[c]["out"]
    return out


def kernel(**inputs):
    nc = _get_nc()
    in_maps = make_in_maps(**inputs)
    res = run_bass_kernel_spmd(nc, in_maps, core_ids=list(range(8)))
    return gather(res.results)


if __name__ == "__main__":
    rng = np.random.default_rng(0)
    ins = {
        "q": rng.standard_normal((2, N, D), dtype=np.float32),
        "kv": rng.standard_normal((2, N, D), dtype=np.float32),
        "Wq": (rng.standard_normal((D, D), dtype=np.float32) / np.sqrt(D)).astype(np.float32),
        "Wkv": (rng.standard_normal((D, 2 * D), dtype=np.float32) / np.sqrt(D)).astype(np.float32),
        "Wout": (rng.standard_normal((D, D), dtype=np.float32) / np.sqrt(D)).astype(np.float32),
    }
    out = kernel(**ins)
    print("ok", out.shape, out.dtype)
